# revision 1
# baseline (speedup 1.0000x reference)
"""GCN layer kernel for Trainium2, 8 NeuronCores (SPMD).

Math (see reference):
    deg = scatter_add(ones, row); deg = max(deg, 1)
    norm_e = rsqrt(deg[row_e]) * rsqrt(deg[col_e])
    agg[row_e] += x[col_e] * norm_e
    out = agg @ W.T + b

Device strategy:
  - Shard DESTINATION nodes across 8 cores (12500 each) -> no collective
    needed; each core computes its own slice of the output.
  - Edges sorted by destination, grouped into windows of 256 destination
    nodes. Per 128-edge tile, gather x[src] rows via the custom dma_gather
    instruction (int16 idxs; x is viewed in 4 chunks of 32768 rows so
    indices fit int16). The scatter-add runs on the TensorEngine:
        aggT[f, d] += msgs[e, f]^T @ onehot[e, d]
    where onehot[e, d] = (d == dloc_e) * norm_e is built by one fused DVE
    tensor_scalar (is_equal, mult) per tile, accumulating in PSUM over the
    window. aggT lands feature-major, which is exactly the lhsT layout the
    dense linear needs:  out[d, o] = aggT[:, d]^T @ W^T[:, o] + b[o].
  - Matmuls run as float32r (full fp32 storage, reduced-precision PE mode,
    ~3e-4 rel err) at 1 cycle/row for N=256.

Host-side work is limited to index preprocessing (sort/shard/pad, int16
tables, degree/norm coefficients) and final unpadding/concat.
"""

import numpy as np
from contextlib import ExitStack

N_NODES = 100000
N_EDGES = 1600000
D = 128
NCORES = 8
NLOC = N_NODES // NCORES          # 12500 real dests per core
WD = 256                          # dest window
NWIN = (NLOC + WD - 1) // WD      # 49 windows (12544 padded dests)
NPAD = NWIN * WD                  # 12544
CHUNK = 32768                     # x chunk rows (int16 index range)
NCHUNK = (N_NODES + CHUNK - 1) // CHUNK  # 4
P = 128


def _host_prep(x, edge_index, W, b):
    """Sort/shard/pad edges; build per-core device arrays.

    Returns (in_maps, T_wc, CW, layout) where T_wc[w][c] is the number of
    128-edge tiles for window w, chunk c (identical across cores)."""
    row = np.asarray(edge_index[0], dtype=np.int64)
    col = np.asarray(edge_index[1], dtype=np.int64)

    deg = np.bincount(row, minlength=N_NODES).astype(np.float32)
    deg = np.maximum(deg, 1.0)
    rs = 1.0 / np.sqrt(deg)
    norm = (rs[row] * rs[col]).astype(np.float32)

    core = row // NLOC
    local = row - core * NLOC
    win = local // WD
    chunk = col >> 15
    key = ((core * NWIN + win) * NCHUNK + chunk).astype(np.int64)
    order = np.argsort(key, kind="stable")
    key_s = key[order]
    col_s = col[order]
    local_s = local[order]
    norm_s = norm[order]

    ngroups = NCORES * NWIN * NCHUNK
    counts = np.bincount(key_s, minlength=ngroups).reshape(NCORES, NWIN, NCHUNK)
    starts = np.zeros(ngroups + 1, dtype=np.int64)
    np.cumsum(counts.reshape(-1), out=starts[1:])

    # tiles per (window, chunk): max over cores, 0 if empty everywhere
    T_wc = np.ceil(counts.max(axis=0) / P).astype(np.int64)  # [NWIN, NCHUNK]
    total_tiles = int(T_wc.sum())
    total_edges_padded = total_tiles * P

    # packed consts layout: [dloc(total_tiles) | nrm(total_tiles) | iota(WD) | brep(D)]
    C_IOTA, C_BREP = 2 * total_tiles, 2 * total_tiles + WD
    CW = 2 * total_tiles + WD + D
    iota = np.tile(np.arange(WD, dtype=np.float32), (P, 1))
    brep = np.tile(np.asarray(b, np.float32).reshape(1, D), (P, 1))
    WT = np.ascontiguousarray(np.asarray(W, np.float32).T)
    x32 = np.ascontiguousarray(np.asarray(x, np.float32))

    idx_cols = total_edges_padded // 16
    in_maps = []
    for k in range(NCORES):
        dl_flat = np.zeros(total_edges_padded, np.float32)
        nm_flat = np.zeros(total_edges_padded, np.float32)
        ix_flat = np.zeros(total_edges_padded, np.int16)
        off = 0
        for w in range(NWIN):
            for c in range(NCHUNK):
                t = int(T_wc[w, c])
                if t == 0:
                    continue
                g = (k * NWIN + w) * NCHUNK + c
                s, e = starts[g], starts[g + 1]
                n = int(e - s)
                sl = slice(off, off + n)
                ix_flat[sl] = (col_s[s:e] - (c << 15)).astype(np.int16)
                dl_flat[sl] = (local_s[s:e] - w * WD).astype(np.float32)
                nm_flat[sl] = norm_s[s:e]
                # pad edges: src 0 (chunk-local row 0), norm 0, dloc 0
                off += t * P
        assert off == total_edges_padded

        # gathered row i of a group lands at msgs[p = i%128, tile i//128]:
        # per-tile column layout for dloc/nrm = [P, tiles]
        dloc2 = dl_flat.reshape(total_tiles, P).T  # [P, total_tiles]
        nrm2 = nm_flat.reshape(total_tiles, P).T
        consts = np.concatenate([dloc2, nrm2, iota, brep], axis=1).astype(np.float32)

        # idx table: wrapped in 16 partitions (i -> [i%16, i//16]),
        # replicated to 128 partitions (8 q7 cores)
        idx16 = np.ascontiguousarray(ix_flat.reshape(idx_cols, 16).T)  # [16, cols]
        idx128 = np.tile(idx16, (8, 1))

        in_maps.append({
            "x": x32,
            "idxs": idx128,
            "consts": consts,
            "wt": WT,
        })

    layout = dict(C_DLOC=0, C_NRM=total_tiles, C_IOTA=C_IOTA, C_BREP=C_BREP,
                  idx_cols=idx_cols)
    return in_maps, T_wc, CW, layout


def _build_nc(T_wc, CW, layout, repeat=1, parts=("gather", "onehot", "matmul", "epilogue"),
              nqueues=1, sp=False, max_idx=0, gdt="f32r", dyn_repeat=False, gbufs=3, ohbufs=6, ohodd=False, ohmode="ts"):
    from concourse import bacc, mybir
    import concourse.tile as tile

    f32 = mybir.dt.float32
    f32r = mybir.dt.float32r
    i16 = mybir.dt.int16

    C_DLOC = layout["C_DLOC"]
    C_NRM = layout["C_NRM"]
    C_IOTA = layout["C_IOTA"]
    C_BREP = layout["C_BREP"]
    idx_cols = layout["idx_cols"]

    T_max = int(T_wc.sum(axis=1).max())

    xdt = mybir.dt.bfloat16 if gdt == "bf16" else f32r
    nc = bacc.Bacc("TRN2", num_swdge_queues=nqueues)
    x_ext = nc.declare_dram_parameter("x", [N_NODES, D], xdt, isOutput=False)
    idx_ext = nc.declare_dram_parameter("idxs", [P, idx_cols], i16, isOutput=False)
    c_ext = nc.declare_dram_parameter("consts", [P, CW], f32, isOutput=False)
    wt_ext = nc.declare_dram_parameter("wt", [D, D], f32r, isOutput=False)
    if dyn_repeat:
        nrep_ext = nc.declare_dram_parameter("nrep", [1, 1], mybir.dt.int32, isOutput=False)
    out_ext = nc.declare_dram_parameter("out", [NPAD, D], f32, isOutput=True)

    with tile.TileContext(nc) as tc:
        with ExitStack() as ctx:
            const = ctx.enter_context(tc.tile_pool(name="const", bufs=1))
            gat = ctx.enter_context(tc.tile_pool(name="gat", bufs=gbufs))
            oh_pool = ctx.enter_context(tc.tile_pool(name="oh", bufs=ohbufs))
            ep = ctx.enter_context(tc.tile_pool(name="ep", bufs=3))
            psum = ctx.enter_context(tc.tile_pool(name="psum", bufs=2, space="PSUM"))
            psum_o = ctx.enter_context(tc.tile_pool(name="psum_o", bufs=2, space="PSUM"))
            psum_c = ctx.enter_context(tc.tile_pool(name="psum_c", bufs=1, space="PSUM"))

            idx_sb = const.tile([P, idx_cols], i16)
            nc.sync.dma_start(idx_sb[:], idx_ext[:])
            c_sb = const.tile([P, CW], f32)
            nc.sync.dma_start(c_sb[:], c_ext[:])
            wt_sb = const.tile([D, D], f32r)
            nc.sync.dma_start(wt_sb[:], wt_ext[:])

            iota_ap = c_sb[:, C_IOTA:C_IOTA + WD]
            brep_ap = c_sb[:, C_BREP:C_BREP + D]
            if ohmode == "psum":
                iota_ps = psum_c.tile([P, WD], f32, space="PSUM")
                nc.vector.tensor_copy(iota_ps[:], c_sb[:, C_IOTA:C_IOTA + WD])
                iota_in = iota_ps
            else:
                iota_in = None

            fake_msgs = None
            if "fakegather" in parts:
                fake_msgs = const.tile([P, T_max * D], xdt)
                nc.sync.dma_start(fake_msgs[:], x_ext[0:P * T_max, :].rearrange("(p t) d -> p (t d)", p=P))

            rep_ctx = None
            if dyn_repeat:
                nrep_sb = const.tile([1, 1], mybir.dt.int32)
                nc.sync.dma_start(nrep_sb[:], nrep_ext[:])
                nrep_val = nc.values_load(nrep_sb[:], min_val=0, max_val=1 << 20)
                rep_ctx = tc.For_i(0, nrep_val, 1)
                rep_ctx.__enter__()

            for _rep in range(repeat):
                tile_off = 0  # global tile counter (column in dloc/nrm)
                for w in range(NWIN):
                    t_win = int(T_wc[w].sum())
                    if t_win == 0:
                        continue
                    if fake_msgs is not None:
                        msgs = fake_msgs
                    else:
                        msgs = gat.tile([P, T_max * D], xdt, tag="msgs")
                    t_in_win = 0
                    for c in range(NCHUNK):
                        t = int(T_wc[w, c])
                        if t == 0:
                            continue
                        n_idx = t * P
                        icol0 = (tile_off + t_in_win) * (P // 16)
                        if "gather" in parts:
                            # optionally split the call into <= max_idx chunks
                            tstep = t if max_idx == 0 else max(1, max_idx // P)
                            for t0 in range(0, t, tstep):
                                tn = min(tstep, t - t0)
                                nc.gpsimd.dma_gather(
                                    out_ap=msgs[:, (t_in_win + t0) * D:(t_in_win + t0 + tn) * D]
                                    .rearrange("p (c d) -> p c d", d=D),
                                    in_ap=x_ext[c * CHUNK:min((c + 1) * CHUNK, N_NODES), :],
                                    idxs_ap=idx_sb[:, icol0 + t0 * (P // 16):icol0 + (t0 + tn) * (P // 16)],
                                    num_idxs=tn * P,
                                    num_idxs_reg=tn * P,
                                    elem_size=D,
                                    single_packet=sp,
                                    queue_num=(w % nqueues),
                                )
                        t_in_win += t

                    aggT_ps = psum.tile([P, WD], f32, space="PSUM")
                    ohw = WD + 1 if ohodd else WD
                    for t in range(t_win):
                        col = tile_off + t
                        oh = oh_pool.tile([P, ohw], f32r)
                        if "onehot" in parts:
                            if ohmode == "psum":
                                nc.vector.tensor_scalar(
                                    out=oh[:, :WD],
                                    in0=iota_ps[:],
                                    scalar1=c_sb[:, C_DLOC + col:C_DLOC + col + 1],
                                    scalar2=c_sb[:, C_NRM + col:C_NRM + col + 1],
                                    op0=mybir.AluOpType.is_equal,
                                    op1=mybir.AluOpType.mult,
                                )
                            elif ohmode == "stt":
                                nc.vector.scalar_tensor_tensor(
                                    out=oh[:, :WD],
                                    in0=c_sb[:, C_IOTA:C_IOTA + WD],
                                    scalar=c_sb[:, C_DLOC + col:C_DLOC + col + 1],
                                    in1=c_sb[:, C_NRM + col:C_NRM + col + 1].to_broadcast((P, WD)),
                                    op0=mybir.AluOpType.is_equal,
                                    op1=mybir.AluOpType.mult,
                                )
                            else:
                                nc.vector.tensor_scalar(
                                    out=oh[:, :ohw],
                                    in0=c_sb[:, C_IOTA:C_IOTA + ohw],
                                    scalar1=c_sb[:, C_DLOC + col:C_DLOC + col + 1],
                                    scalar2=c_sb[:, C_NRM + col:C_NRM + col + 1],
                                    op0=mybir.AluOpType.is_equal,
                                    op1=mybir.AluOpType.mult,
                                )
                        if "matmul" in parts:
                            nc.tensor.matmul(
                                out=aggT_ps[:],
                                lhsT=msgs[:, t * D:(t + 1) * D],
                                rhs=oh[:, :WD],
                                start=(t == 0),
                                stop=(t == t_win - 1),
                            )

                    if "epilogue" not in parts or "matmul" not in parts:
                        tile_off += t_win
                        continue
                    aggT_sb = ep.tile([P, WD], f32r, tag="aggT")
                    nc.scalar.copy(aggT_sb[:], aggT_ps[:])
                    for h in range(WD // P):
                        out_ps = psum_o.tile([P, D], f32, space="PSUM")
                        nc.tensor.matmul(
                            out=out_ps[:],
                            lhsT=aggT_sb[:, h * P:(h + 1) * P],
                            rhs=wt_sb[:],
                            start=True, stop=True,
                        )
                        out_sb = ep.tile([P, D], f32, tag="out")
                        nc.vector.tensor_tensor(
                            out=out_sb[:],
                            in0=out_ps[:],
                            in1=brep_ap,
                            op=mybir.AluOpType.add,
                        )
                        nc.sync.dma_start(
                            out_ext[w * WD + h * P: w * WD + (h + 1) * P, :],
                            out_sb[:],
                        )
                    tile_off += t_win

            if rep_ctx is not None:
                rep_ctx.__exit__(None, None, None)

    nc.compile()
    return nc


def run(x, edge_index, W, b, trace=False):
    """Build + run on 8 cores. Returns (out, results)."""
    from concourse.bass_utils import run_bass_kernel_spmd

    in_maps, T_wc, CW, layout = _host_prep(x, edge_index, W, b)
    nc = _build_nc(T_wc, CW, layout, nqueues=4, ohmode="psum")
    res = run_bass_kernel_spmd(nc, in_maps, list(range(NCORES)), trace=trace)
    parts = [res.results[k]["out"][:NLOC] for k in range(NCORES)]
    out = np.concatenate(parts, axis=0).astype(np.float32)
    return out, res


def kernel(x, edge_index, W, b):
    out, _ = run(x, edge_index, W, b)
    return out


# ---------------------------------------------------------------------------
# benchmarking: time repeat=R vs repeat=1 NEFFs with device-resident inputs;
# the delta cancels transfers/dispatch and yields per-iteration HW time.
# ---------------------------------------------------------------------------

def _make_callable(nc, in_maps):
    import jax
    import numpy as _np
    from jax.sharding import Mesh, PartitionSpec, NamedSharding
    from jax.experimental.shard_map import shard_map
    from concourse import mybir
    from concourse.bass2jax import (
        _bass_exec_p, install_neuronx_cc_hook, partition_id_tensor,
    )

    install_neuronx_cc_hook()
    n_cores = len(in_maps)
    in_names, out_names, out_avals, zero_outs = [], [], [], []
    for alloc in nc.m.functions[0].allocations:
        if not isinstance(alloc, mybir.MemoryLocationSet):
            continue
        name = alloc.memorylocations[0].name
        if alloc.kind == "ExternalInput":
            if nc.partition_id_tensor is None or name != nc.partition_id_tensor.name:
                in_names.append(name)
        elif alloc.kind == "ExternalOutput":
            out_names.append(name)
            shape = tuple(alloc.tensor_shape)
            dtype = mybir.dt.np(alloc.dtype)
            out_avals.append(jax.core.ShapedArray(shape, dtype))
            zero_outs.append(_np.zeros(shape, dtype))
    n_params = len(in_names)
    all_in_names = in_names + out_names
    if nc.partition_id_tensor is not None:
        all_in_names = all_in_names + [nc.partition_id_tensor.name]

    def _body(*args):
        operands = list(args)
        if nc.partition_id_tensor is not None:
            operands.append(partition_id_tensor())
        outs = _bass_exec_p.bind(
            *operands,
            out_avals=tuple(out_avals),
            in_names=tuple(all_in_names),
            out_names=tuple(out_names),
            lowering_input_output_aliases=(),
            sim_require_finite=True,
            sim_require_nnan=True,
            nc=nc,
        )
        return tuple(outs)

    devices = jax.devices()[:n_cores]
    mesh = Mesh(_np.asarray(devices), ("core",))
    spec = PartitionSpec("core")
    in_specs = (spec,) * (n_params + len(out_names))
    out_specs = (spec,) * len(out_names)
    fn = jax.jit(shard_map(_body, mesh=mesh, in_specs=in_specs,
                           out_specs=out_specs, check_rep=False),
                 keep_unused=True)
    sharding = NamedSharding(mesh, spec)
    dev_in = [
        jax.device_put(
            _np.concatenate([_np.asarray(in_maps[c][nm]) for c in range(n_cores)], axis=0),
            sharding)
        for nm in in_names
    ]
    dev_zero = [
        jax.device_put(_np.zeros((n_cores * z.shape[0], *z.shape[1:]), z.dtype), sharding)
        for z in zero_outs
    ]
    return fn, dev_in, dev_zero


def bench(x, edge_index, W, b, big_repeat=5, iters=4):
    import time
    import jax

    in_maps, T_wc, CW, layout = _host_prep(x, edge_index, W, b)
    times = {}
    for R in (1, big_repeat):
        nc = _build_nc(T_wc, CW, layout, repeat=R)
        fn, dev_in, dev_zero = _make_callable(nc, in_maps)
        outs = fn(*dev_in, *dev_zero)  # compile + warm
        jax.block_until_ready(outs)
        best = float("inf")
        for _ in range(iters):
            t0 = time.perf_counter()
            outs = fn(*dev_in, *dev_zero)
            jax.block_until_ready(outs)
            best = min(best, time.perf_counter() - t0)
        times[R] = best
        print(f"repeat={R}: best wall {best*1e3:.3f} ms")
    per_iter_ns = (times[big_repeat] - times[1]) / (big_repeat - 1) * 1e9
    return per_iter_ns, times

